# revision 1
# baseline (speedup 1.0000x reference)
"""Trainium2 Bass kernel for CropConLoss (supervised-contrastive style loss).

Contract: kernel(**inputs) takes the FULL unsharded inputs
(protos [64,128] f32, proj2/proj3 [4096,128] f32, target2/target3 [4096] i64)
and returns the FULL output (scalar f32 mean loss), running the compute on
8 NeuronCores via bass_utils.run_bass_kernel_spmd.

Strategy (data-parallel over the M=8192 rows of feats):
  - Each core owns 1024 query rows. The host hands every core a np.roll'd
    copy of all 8192 feature rows (its own queries first), so the
    diagonal-masking control flow is identical on every core (SPMD-safe).
  - Per core: sim tile [128 keys, 1024 q] = keysT_kt^T @ qnT via PE;
    exp via ACT with the per-key 1/(tau*|x_k|) folded into the activation
    scale (so keys never need explicit normalization); per-class sums +
    row sum accumulated with a second matmul (one-hot-augmented stationary)
    into a persistent PSUM accumulator [65+, 1024].
  - Epilogue selects numer (own-class sum + proto term) and denom
    (weighted row-sum + freq-weighted proto sums) with one-hot masks and
    ones-matmul partition reductions, then ACT Ln with fused free-dim
    accumulation; each core returns sum(loss_rows) over its 1024 rows.
  - Host sums the 8 partials and divides by 8192. No device collectives.
"""

import sys
import types

sys.path.insert(0, "/opt/trn_rl_repo")

import numpy as np

TAU = 0.1
EPS_FREQ = 1e-06
EPS_DENOM = 1e-12

N_CORES = 8
M = 8192          # total rows (2*4096)
D = 128           # feature dim
C = 64            # num classes
Q = M // N_CORES  # 1024 query rows per core
NT = M // 128     # 64 key tiles of 128


def _install_ntff_hook():
    """Shim antenv.axon_hooks (absent in this image) so trace=True works."""
    if "antenv.axon_hooks" in sys.modules:
        return
    try:
        if "/root/.axon_site" not in sys.path:
            sys.path.insert(0, "/root/.axon_site")
        import trn_agent_boot.trn_boot as tb

        hook = tb._ntff_profile_via_ctypes("/opt/axon/libaxon_pjrt.so")
        mod = types.ModuleType("antenv.axon_hooks")
        mod._hook = hook
        mod.get_axon_ntff_profile_hook = lambda: mod._hook
        mod.set_axon_ntff_profile_hook = lambda h: setattr(mod, "_hook", h)
        sys.modules["antenv.axon_hooks"] = mod
        import antenv

        antenv.axon_hooks = mod
    except Exception:
        pass


def build_nc(n_kt=NT, do_epi=True, do_main=True):
    """Build and compile the single-core Bass program (same NEFF on all 8)."""
    import concourse.bass as bass  # noqa: F401
    import concourse.mybir as mybir
    import concourse.bacc as bacc
    from concourse import tile

    f32 = mybir.dt.float32
    bf16 = mybir.dt.bfloat16
    mult = mybir.AluOpType.mult
    add = mybir.AluOpType.add
    Act = mybir.ActivationFunctionType

    nc = bacc.Bacc("TRN2", target_bir_lowering=False, debug=False,
                   num_devices=N_CORES)

    # DRAM I/O (per-core data is provided via in_maps)
    d_keysT = nc.dram_tensor("keysT", [128, M], bf16, kind="ExternalInput")
    d_keysN = nc.dram_tensor("keysN", [128, NT, 128], bf16, kind="ExternalInput")
    d_onehot = nc.dram_tensor("onehot", [128, NT, 128], bf16, kind="ExternalInput")
    d_mask = nc.dram_tensor("mask8", [128, 8, Q], bf16, kind="ExternalInput")
    d_ohqT = nc.dram_tensor("ohqT", [C + 1, Q], f32, kind="ExternalInput")
    d_fwinv = nc.dram_tensor("fwinv", [1, Q], f32, kind="ExternalInput")
    d_cfinv = nc.dram_tensor("cfinv", [C + 1, 1], f32, kind="ExternalInput")
    d_ones = nc.dram_tensor("ones65", [C + 1, 1], f32, kind="ExternalInput")
    d_ident = nc.dram_tensor("ident", [128, 128], bf16, kind="ExternalInput")
    d_protos = nc.dram_tensor("protos", [C, 128], f32, kind="ExternalInput")
    d_out = nc.dram_tensor("out", [1, 1], f32, kind="ExternalOutput")

    with tile.TileContext(nc) as tc:
        with (
            tc.tile_pool(name="const", bufs=1) as cst,
            tc.tile_pool(name="work", bufs=3) as work,
        ):
            # ---- resident SBUF tensors ----
            keysT = cst.tile([128, M], bf16, tag="keysT")
            keysN = cst.tile([128, NT, 128], bf16, tag="keysN")
            onehot = cst.tile([128, NT, 128], bf16, tag="onehot")
            mask8 = cst.tile([128, 8, Q], bf16, tag="mask8")
            ohqT = cst.tile([C + 1, Q], f32, tag="ohqT")
            fwinv = cst.tile([1, Q], f32, tag="fwinv")
            cfinv = cst.tile([C + 1, 1], f32, tag="cfinv")
            ones65 = cst.tile([C + 1, 1], f32, tag="ones65")
            ident = cst.tile([128, 128], bf16, tag="ident")
            protos = cst.tile([C, 128], f32, tag="protos")

            nc.sync.dma_start(keysN[:], d_keysN[:])
            nc.sync.dma_start(keysT[:], d_keysT[:])
            nc.sync.dma_start(onehot[:], d_onehot[:])
            nc.sync.dma_start(mask8[:], d_mask[:])
            nc.sync.dma_start(ohqT[:], d_ohqT[:])
            nc.sync.dma_start(fwinv[:], d_fwinv[:])
            nc.sync.dma_start(cfinv[:], d_cfinv[:])
            nc.sync.dma_start(ones65[:], d_ones[:])
            nc.sync.dma_start(ident[:], d_ident[:])
            nc.sync.dma_start(protos[:], d_protos[:])

            ss = cst.tile([128, NT], f32, tag="ss")       # per-key |x|^2
            srt = cst.tile([128, NT], f32, tag="srt")     # |x|
            rinv = cst.tile([128, NT], f32, tag="rinv")   # 1/|x|
            rinv10 = cst.tile([128, NT], f32, tag="rinv10")  # (1/tau)/|x|
            qnT = cst.tile([128, Q], bf16, tag="qnT")     # normalized queries, [d, q]
            protosT = cst.tile([128, C + 1], bf16, tag="protosT")
            p_t = cst.tile([C + 1, Q], f32, tag="p_t")    # exp(proto_sim/tau)

            # ---- prologue ----
            with (
                tc.tile_pool(name="pA", bufs=2, space="PSUM") as pA,
                tc.tile_pool(name="pB", bufs=1, space="PSUM") as pB,
            ):
                # per-key sum of squares -> |x| -> 1/|x|
                for rt in range(NT):
                    sq = work.tile([128, 128], f32, tag="sq")
                    nc.vector.tensor_tensor(sq[:], keysN[:, rt], keysN[:, rt],
                                            op=mult)
                    nc.vector.reduce_sum(ss[:, rt:rt + 1], sq[:],
                                         axis=mybir.AxisListType.X)
                nc.scalar.activation(srt[:], ss[:], Act.Sqrt)
                nc.vector.reciprocal(rinv[:], srt[:])
                nc.vector.tensor_scalar_mul(rinv10[:], rinv[:], 1.0 / TAU)

                # normalize own 8 query tiles, transpose into qnT [d, q]
                for t in range(8):
                    qn = work.tile([128, 128], bf16, tag="qn")
                    nc.vector.tensor_scalar_mul(qn[:], keysN[:, t],
                                                rinv[:, t:t + 1])
                    tp = pA.tile([128, 128], bf16, tag="tp")
                    nc.tensor.transpose(tp[:], qn[:], ident[:])
                    nc.vector.tensor_copy(qnT[:, t * 128:(t + 1) * 128], tp[:])

                # normalize protos, transpose into protosT cols 1..64
                psq = work.tile([C, 128], f32, tag="psq")
                ssp = work.tile([C, 1], f32, tag="ssp")
                nc.vector.tensor_tensor(psq[:], protos[:], protos[:], op=mult)
                nc.vector.reduce_sum(ssp[:], psq[:],
                                     axis=mybir.AxisListType.X)
                srtp = work.tile([C, 1], f32, tag="srtp")
                nc.scalar.activation(srtp[:], ssp[:], Act.Sqrt)
                rinvp = work.tile([C, 1], f32, tag="rinvp")
                nc.vector.reciprocal(rinvp[:], srtp[:])
                pn = work.tile([C, 128], bf16, tag="pn")
                nc.vector.tensor_scalar_mul(pn[:], protos[:], rinvp[:])
                ptp = pA.tile([128, C], bf16, tag="ptp")
                nc.tensor.transpose(ptp[:], pn[:], ident[0:C, 0:C])
                nc.vector.memset(protosT[:, 0:1], 0.0)
                nc.vector.tensor_copy(protosT[:, 1:C + 1], ptp[:])

                # proto similarities for own queries: [65, 1024]
                pp = pB.tile([C + 1, Q], f32, tag="pp")
                for j in range(Q // 512):
                    nc.tensor.matmul(pp[:, j * 512:(j + 1) * 512],
                                     protosT[:], qnT[:, j * 512:(j + 1) * 512],
                                     start=True, stop=True)
                nc.scalar.activation(p_t[:], pp[:], Act.Exp, scale=1.0 / TAU)

            # ---- main loop over 64 key tiles ----
            with tc.tile_pool(name="acc", bufs=1, space="PSUM") as acc:
                sT = acc.tile([128, Q], f32, tag="sT")
                with tc.tile_pool(name="ring", bufs=3, space="PSUM") as ring:
                    exp_tiles = {}
                    for kt in range(n_kt if do_main else 0):
                        ps = ring.tile([128, Q], f32, tag="ps")
                        for j in range(Q // 512):
                            nc.tensor.matmul(
                                ps[:, j * 512:(j + 1) * 512],
                                keysT[:, kt * 128:(kt + 1) * 128],
                                qnT[:, j * 512:(j + 1) * 512],
                                start=True, stop=True)
                        # software-pipelined: class-sum matmul for kt-1
                        if kt > 0:
                            et_p = exp_tiles.pop(kt - 1)
                            for j in range(Q // 512):
                                nc.tensor.matmul(
                                    sT[:, j * 512:(j + 1) * 512],
                                    onehot[:, kt - 1],
                                    et_p[:, j * 512:(j + 1) * 512],
                                    start=(kt - 1 == 0), stop=False)
                        et = work.tile([128, Q], bf16, tag="et")
                        nc.scalar.activation(et[:], ps[:], Act.Exp,
                                             scale=rinv10[:, kt:kt + 1])
                        if kt < 8:
                            nc.vector.tensor_tensor(et[:], et[:], mask8[:, kt],
                                                    op=mult)
                        exp_tiles[kt] = et
                    if do_main:
                        et_p = exp_tiles.pop(n_kt - 1)
                        for j in range(Q // 512):
                            nc.tensor.matmul(
                                sT[:, j * 512:(j + 1) * 512],
                                onehot[:, n_kt - 1],
                                et_p[:, j * 512:(j + 1) * 512],
                                start=(n_kt == 1), stop=True)
                    else:
                        nc.vector.memset(sT[:], 0.0)
                        zz = work.tile([128, Q], f32, tag="zz")
                        nc.vector.tensor_copy(zz[:], sT[:])

                # ---- epilogue ----
                if do_epi:
                  with tc.tile_pool(name="epi", bufs=1, space="PSUM") as epi:
                    # b[m,q] = (S_T + P_T) * onehotQ ; row0 zeroed via ohqT
                    b = cst.tile([C + 1, Q], f32, tag="b")
                    nc.vector.tensor_tensor(b[:], sT[0:C + 1, :], p_t[:], op=add)
                    nc.vector.tensor_tensor(b[:], b[:], ohqT[:], op=mult)
                    # c2[m,q] = P_T * (1/cls_freq[c]) ; row0 zeroed via cfinv
                    c2 = cst.tile([C + 1, Q], f32, tag="c2")
                    nc.vector.tensor_scalar_mul(c2[:], p_t[:], cfinv[:])

                    pn_ = epi.tile([1, Q], f32, tag="pnum")
                    pd_ = epi.tile([1, Q], f32, tag="pden")
                    for j in range(Q // 512):
                        nc.tensor.matmul(pn_[:, j * 512:(j + 1) * 512],
                                         ones65[:], b[:, j * 512:(j + 1) * 512],
                                         start=True, stop=True)
                        nc.tensor.matmul(pd_[:, j * 512:(j + 1) * 512],
                                         ones65[:], c2[:, j * 512:(j + 1) * 512],
                                         start=True, stop=True)

                    # denom = rowsum/feat_w + denom_proto + eps
                    den = cst.tile([1, Q], f32, tag="den")
                    nc.vector.tensor_tensor(den[:], sT[0:1, :], fwinv[:], op=mult)
                    nc.vector.tensor_tensor(den[:], den[:], pd_[:], op=add)
                    nc.vector.tensor_scalar_add(den[:], den[:], EPS_DENOM)

                    lbuf = cst.tile([1, Q], f32, tag="lbuf")
                    ld_s = cst.tile([1, 1], f32, tag="ld_s")
                    ln_s = cst.tile([1, 1], f32, tag="ln_s")
                    nc.scalar.activation(lbuf[:], den[:], Act.Ln,
                                         accum_out=ld_s[:])
                    lbuf2 = cst.tile([1, Q], f32, tag="lbuf2")
                    nc.scalar.activation(lbuf2[:], pn_[:], Act.Ln,
                                         accum_out=ln_s[:])
                    res = cst.tile([1, 1], f32, tag="res")
                    nc.vector.tensor_tensor(res[:], ld_s[:], ln_s[:],
                                            op=mybir.AluOpType.subtract)
                    nc.sync.dma_start(d_out[:], res[:])
                else:
                    res = cst.tile([1, 1], f32, tag="res")
                    nc.vector.tensor_copy(res[:], sT[0:1, 0:1])
                    nc.sync.dma_start(d_out[:], res[:])

    nc.compile()
    return nc


def make_in_maps(protos, proj2, target2, proj3, target3):
    import ml_dtypes

    bf16 = ml_dtypes.bfloat16
    f32 = np.float32

    feats = np.concatenate([np.asarray(proj2, dtype=f32),
                            np.asarray(proj3, dtype=f32)], axis=0)
    labels = np.concatenate([np.asarray(target2), np.asarray(target3)],
                            axis=0).astype(np.int64)

    counts = np.bincount(labels, minlength=C).astype(f32)
    cls_freq = (counts + f32(1.0)) + f32(EPS_FREQ)   # matches reference f32 math
    cfr = (f32(1.0) / cls_freq).astype(f32)

    # globals (identical on every core)
    mask = np.ones((128, 8, Q), dtype=bf16)
    k_idx = np.arange(128)
    for t in range(8):
        mask[k_idx, t, t * 128 + k_idx] = bf16(0.0)
    ident = np.eye(128, dtype=bf16)
    cfinv = np.zeros((C + 1, 1), dtype=f32)
    cfinv[1:, 0] = cfr
    ones65 = np.ones((C + 1, 1), dtype=f32)
    protos_f = np.ascontiguousarray(np.asarray(protos, dtype=f32))

    in_maps = []
    for c in range(N_CORES):
        idx = (np.arange(M) + c * Q) % M
        kf = feats[idx]                      # [8192, 128] rolled
        kl = labels[idx]

        keysT = np.ascontiguousarray(kf.T).astype(bf16)          # [128, 8192]
        keysN = np.ascontiguousarray(
            kf.reshape(NT, 128, 128).transpose(1, 0, 2)).astype(bf16)

        oh = np.zeros((M, 128), dtype=bf16)
        oh[np.arange(M), 1 + kl] = bf16(1.0)   # cols 1..64 = class indicator
        oh[:, 0] = bf16(1.0)                   # col 0 = row-sum
        onehot = np.ascontiguousarray(
            oh.reshape(NT, 128, 128).transpose(1, 0, 2))

        ohqT = np.zeros((C + 1, Q), dtype=f32)
        ohqT[1 + kl[:Q], np.arange(Q)] = f32(1.0)

        fwinv = cfr[kl[:Q]].reshape(1, Q).astype(f32)

        in_maps.append({
            "keysT": keysT,
            "keysN": keysN,
            "onehot": onehot,
            "mask8": mask,
            "ohqT": ohqT,
            "fwinv": np.ascontiguousarray(fwinv),
            "cfinv": cfinv,
            "ones65": ones65,
            "ident": ident,
            "protos": protos_f,
        })
    return in_maps


def run(in_maps, trace=False):
    _install_ntff_hook()
    from concourse import bass_utils

    nc = build_nc()
    res = bass_utils.run_bass_kernel_spmd(
        nc, in_maps, core_ids=list(range(N_CORES)), trace=trace)
    return res


def kernel(protos, proj2, target2, proj3, target3):
    in_maps = make_in_maps(protos, proj2, target2, proj3, target3)
    res = run(in_maps, trace=False)
    parts = [res.results[i]["out"][0, 0] for i in range(N_CORES)]
    total = np.sum(np.asarray(parts, dtype=np.float32))
    return np.asarray(total / np.float32(M), dtype=np.float32)



# revision 6
# speedup vs baseline: 1.4084x; 1.4084x over previous
"""Trainium2 Bass kernel for CropConLoss (supervised-contrastive style loss).

Contract: kernel(**inputs) takes the FULL unsharded inputs
(protos [64,128] f32, proj2/proj3 [4096,128] f32, target2/target3 [4096] i64)
and returns the FULL output (scalar f32 mean loss), running the compute on
8 NeuronCores via bass_utils.run_bass_kernel_spmd.

Strategy (v2 — query-partition layout, ACT-roofline design):
  - Host: L2-normalize feats+protos in f32, SORT the 8192 rows by label.
    Core c owns sorted rows [c*1024, (c+1)*1024) as queries. Each core gets
    a cyclically rolled copy of the normalized keys (bf16, [128d x 8192k])
    with its own queries at columns 256..1280, so all same-class keys of
    query (t, p) lie inside the fixed window [t*128, t*128+768) — identical
    control flow on every core (SPMD-safe), per-core data in in_maps.
  - Device main loop: sim tile [128q, 2048k] = qT_t^T @ keysT chunks via PE
    (4 x 512-col matmuls into a ping-pong PSUM pair); ACT computes
    et = exp(10*sim) back-to-back (the critical path); DVE reduces each et
    tile to a denominator row-partial (tensor_scalar copy w/ accum_out, 4x
    mode) and, for the first column pair only, the numerator via a masked
    768-wide tensor_tensor_reduce (mask excludes self; built on host).
  - Self-similarity term is removed by subtracting exp(10) from the row sum
    (keys are pre-normalized, so sim_ii = 1 up to bf16 rounding).
  - Proto terms: [128q, 64c] matmul + exp; masked reduces give
    numer_proto (onehot) and denom_proto (1/cls_freq weights).
  - Epilogue: denom = (rowsum - e^10)*fwinv + denom_proto + eps;
    loss rows = ln(denom) - ln(numer); free-dim accum + ones-matmul
    partition reduce -> scalar partial per core. Host sums 8 partials /8192.
"""

import sys
import types

sys.path.insert(0, "/opt/trn_rl_repo")

import numpy as np

TAU = 0.1
EPS_FREQ = 1e-06
EPS_DENOM = 1e-12

N_CORES = 8
M = 8192           # total rows (2*4096)
D = 128            # feature dim
C = 64             # num classes
Q = M // N_CORES   # 1024 query rows per core
QT = Q // 128      # 8 query tiles per core
NPAIR = 4          # 4 column groups of 2048 keys
WIN = 768          # numer window width (covers class runs up to 257)
OWN_OFF = 256      # own queries start at this column of the rolled buffer
E10 = float(np.exp(np.float64(1.0 / TAU)))  # 22026.4657948...


def _install_ntff_hook():
    """Shim antenv.axon_hooks (absent in this image) so trace=True works."""
    if "antenv.axon_hooks" in sys.modules:
        return
    try:
        if "/root/.axon_site" not in sys.path:
            sys.path.insert(0, "/root/.axon_site")
        import trn_agent_boot.trn_boot as tb

        hook = tb._ntff_profile_via_ctypes("/opt/axon/libaxon_pjrt.so")
        mod = types.ModuleType("antenv.axon_hooks")
        mod._hook = hook
        mod.get_axon_ntff_profile_hook = lambda: mod._hook
        mod.set_axon_ntff_profile_hook = lambda h: setattr(mod, "_hook", h)
        sys.modules["antenv.axon_hooks"] = mod
        import antenv

        antenv.axon_hooks = mod
    except Exception:
        pass


def build_nc():
    """Build and compile the single-core Bass program (same NEFF on all 8)."""
    import concourse.bass as bass  # noqa: F401
    import concourse.mybir as mybir
    import concourse.bacc as bacc
    from concourse import tile

    f32 = mybir.dt.float32
    bf16 = mybir.dt.bfloat16
    mult = mybir.AluOpType.mult
    add = mybir.AluOpType.add
    sub = mybir.AluOpType.subtract
    Act = mybir.ActivationFunctionType

    nc = bacc.Bacc("TRN2", target_bir_lowering=False, debug=False,
                   num_devices=N_CORES)

    # DRAM I/O (per-core data via in_maps). keysT split in 4 column groups
    # so the first matmuls depend only on the first 512KB DMA.
    d_keys = [nc.dram_tensor(f"keys{p}", [128, 2048], bf16,
                             kind="ExternalInput") for p in range(NPAIR)]
    d_wmask = nc.dram_tensor("wmask", [128, QT, WIN], bf16,
                             kind="ExternalInput")
    d_protosT = nc.dram_tensor("protosT", [128, C], bf16,
                               kind="ExternalInput")
    d_pcls = nc.dram_tensor("pcls", [128, QT, C], bf16, kind="ExternalInput")
    d_cfr = nc.dram_tensor("cfrT", [128, C], f32, kind="ExternalInput")
    d_fwinv = nc.dram_tensor("fwinv", [128, QT], f32, kind="ExternalInput")
    d_ones = nc.dram_tensor("ones1", [128, 1], f32, kind="ExternalInput")
    d_out = nc.dram_tensor("out", [1, 1], f32, kind="ExternalOutput")

    with tile.TileContext(nc) as tc:
        with (
            tc.tile_pool(name="const", bufs=1) as cst,
            tc.tile_pool(name="work", bufs=3) as work,
            tc.tile_pool(name="etring", bufs=4) as etring,
            tc.tile_pool(name="dscr", bufs=2) as dscr,
            tc.tile_pool(name="wscr", bufs=2) as wscr,
        ):
            # ---- resident SBUF tensors ----
            keys = [cst.tile([128, 2048], bf16, name=f"keys_sb{p}", tag=f"keys{p}")
                    for p in range(NPAIR)]
            wmask = cst.tile([128, QT, WIN], bf16, tag="wmask")
            protosT = cst.tile([128, C], bf16, tag="protosT")
            pcls = cst.tile([128, QT, C], bf16, tag="pcls")
            cfrT = cst.tile([128, C], f32, tag="cfrT")
            fwinv = cst.tile([128, QT], f32, tag="fwinv")
            ones1 = cst.tile([128, 1], f32, tag="ones1")

            npro = cst.tile([128, QT], f32, tag="npro")
            dpro = cst.tile([128, QT], f32, tag="dpro")
            nmr = cst.tile([128, QT], f32, tag="nmr")
            accs = [cst.tile([128, QT], f32, name=f"acc_sb{p}", tag=f"acc{p}")
                    for p in range(NPAIR)]

            # ---- DMAs in priority order ----
            nc.sync.dma_start(keys[0][:], d_keys[0][:])
            nc.sync.dma_start(protosT[:], d_protosT[:])
            nc.sync.dma_start(pcls[:], d_pcls[:])
            nc.sync.dma_start(cfrT[:], d_cfr[:])
            nc.sync.dma_start(fwinv[:], d_fwinv[:])
            nc.sync.dma_start(ones1[:], d_ones[:])
            nc.sync.dma_start(wmask[:], d_wmask[:])
            for p in range(1, NPAIR):
                nc.sync.dma_start(keys[p][:], d_keys[p][:])

            # Force the natural_log_exp table set early (serves Exp AND Ln)
            lntmp = work.tile([128, 1], f32, tag="lntmp")
            nc.scalar.activation(lntmp[:], ones1[:], Act.Ln)

            def qstat(t):
                # stationary for q-tile t: own queries at cols 256..1280
                lo = OWN_OFF + t * 128
                return keys[0][:, lo:lo + 128]

            # ---- proto phase: [128q, 64c] per q-tile ----
            with tc.tile_pool(name="pp", bufs=2, space="PSUM") as ppp:
                for t in range(QT):
                    pp = ppp.tile([128, C], f32, tag="pp")
                    nc.tensor.matmul(pp[:], qstat(t), protosT[:],
                                     start=True, stop=True)
                    pe = work.tile([128, C], f32, tag="pe")
                    nc.scalar.activation(pe[:], pp[:], Act.Exp, scale=1.0 / TAU)
                    ws = work.tile([128, C], f32, tag="ws")
                    nc.vector.tensor_tensor(ws[:], pe[:], pcls[:, t], op=mult)
                    wsa = work.tile([128, C], f32, tag="wsa")
                    nc.vector.tensor_scalar(wsa[:], ws[:], 1.0, None,
                                            op0=mult, op1=add,
                                            accum_out=npro[:, t:t + 1])
                    ws2 = work.tile([128, C], f32, tag="ws2")
                    nc.vector.tensor_tensor(ws2[:], pe[:], cfrT[:], op=mult)
                    ws2a = work.tile([128, C], f32, tag="ws2a")
                    nc.vector.tensor_scalar(ws2a[:], ws2[:], 1.0, None,
                                            op0=mult, op1=add,
                                            accum_out=dpro[:, t:t + 1])

            # ---- main loop: 4 column pairs x 8 q-tiles ----
            with tc.tile_pool(name="ps", bufs=2, space="PSUM") as psp:
                for p in range(NPAIR):
                    for t in range(QT):
                        ps = psp.tile([128, 2048], f32, tag="ps")
                        for j in range(4):
                            nc.tensor.matmul(
                                ps[:, j * 512:(j + 1) * 512], qstat(t),
                                keys[p][:, j * 512:(j + 1) * 512],
                                start=True, stop=True)
                        et = etring.tile([128, 2048], bf16, tag="et")
                        nc.scalar.activation(et[:], ps[:], Act.Exp,
                                             scale=1.0 / TAU)
                        # denominator row-partial: copy w/ accumulate (DVE 4x)
                        dsc = dscr.tile([128, 2048], bf16, tag="dsc")
                        nc.vector.tensor_scalar(
                            dsc[:], et[:], 1.0, None, op0=mult, op1=add,
                            accum_out=accs[p][:, t:t + 1])
                        if p == 0:
                            # numerator: masked window reduce (2 DVE ops)
                            wsc = wscr.tile([128, WIN], bf16, tag="wsc")
                            nc.vector.tensor_tensor(
                                wsc[:], et[:, t * 128:t * 128 + WIN],
                                wmask[:, t], op=mult)
                            wsc2 = wscr.tile([128, WIN], bf16, tag="wsc2")
                            nc.vector.tensor_scalar(
                                wsc2[:], wsc[:], 1.0, None, op0=mult,
                                op1=add, accum_out=nmr[:, t:t + 1])

            # ---- epilogue ----
            den = cst.tile([128, QT], f32, tag="den")
            t01 = work.tile([128, QT], f32, tag="t01")
            nc.vector.tensor_tensor(t01[:], accs[0][:], accs[1][:], op=add)
            t23 = work.tile([128, QT], f32, tag="t23")
            nc.vector.tensor_tensor(t23[:], accs[2][:], accs[3][:], op=add)
            nc.vector.tensor_tensor(den[:], t01[:], t23[:], op=add)
            # remove self term, apply 1/feat_w, add proto denom + eps
            nc.vector.tensor_scalar_add(den[:], den[:], -E10)
            nc.vector.tensor_tensor(den[:], den[:], fwinv[:], op=mult)
            nc.vector.tensor_tensor(den[:], den[:], dpro[:], op=add)
            nc.vector.tensor_scalar_add(den[:], den[:], EPS_DENOM)
            nc.vector.tensor_tensor(nmr[:], nmr[:], npro[:], op=add)

            lden = work.tile([128, QT], f32, tag="lden")
            ld = cst.tile([128, 1], f32, tag="ld")
            nc.scalar.activation(lden[:], den[:], Act.Ln, accum_out=ld[:])
            lnum = work.tile([128, QT], f32, tag="lnum")
            ln_ = cst.tile([128, 1], f32, tag="ln_")
            nc.scalar.activation(lnum[:], nmr[:], Act.Ln, accum_out=ln_[:])
            diff = cst.tile([128, 1], f32, tag="diff")
            nc.vector.tensor_tensor(diff[:], ld[:], ln_[:], op=sub)

            with tc.tile_pool(name="rp", bufs=1, space="PSUM") as rp:
                res_ps = rp.tile([1, 1], f32, tag="res_ps")
                nc.tensor.matmul(res_ps[:], ones1[:], diff[:],
                                 start=True, stop=True)
                res = cst.tile([1, 1], f32, tag="res")
                nc.vector.tensor_copy(res[:], res_ps[:])
                nc.sync.dma_start(d_out[:], res[:])

    nc.compile()
    return nc


def make_in_maps(protos, proj2, target2, proj3, target3):
    import ml_dtypes

    bf16 = ml_dtypes.bfloat16
    f32 = np.float32

    feats = np.concatenate([np.asarray(proj2, dtype=f32),
                            np.asarray(proj3, dtype=f32)], axis=0)
    labels = np.concatenate([np.asarray(target2), np.asarray(target3)],
                            axis=0).astype(np.int64)

    # f32 normalization (matches reference F.normalize)
    nrm = np.sqrt(np.sum(feats.astype(f32) ** 2, axis=1, keepdims=True,
                         dtype=f32))
    fn = feats / np.maximum(nrm, f32(1e-12))
    pr = np.asarray(protos, dtype=f32)
    pnrm = np.sqrt(np.sum(pr ** 2, axis=1, keepdims=True, dtype=f32))
    pn = pr / np.maximum(pnrm, f32(1e-12))

    counts = np.bincount(labels, minlength=C).astype(f32)
    cls_freq = (counts + f32(1.0)) + f32(EPS_FREQ)
    cfr = (f32(1.0) / cls_freq).astype(f32)

    perm = np.argsort(labels, kind="stable")
    sf = np.ascontiguousarray(fn[perm])          # [8192, 128] sorted by label
    sl = labels[perm]                            # [8192]
    assert counts.max() <= 257, f"class run too long: {counts.max()}"

    keysT_g = np.ascontiguousarray(sf.T).astype(bf16)   # [128, 8192]
    protosT = np.ascontiguousarray(pn.T).astype(bf16)   # [128, 64]
    cfrT = np.broadcast_to(cfr[None, :], (128, C)).astype(f32).copy()
    ones1 = np.ones((128, 1), dtype=f32)

    in_maps = []
    for c in range(N_CORES):
        qs = c * Q
        roll = (qs - OWN_OFF) % M
        keysT = np.roll(keysT_g, -roll, axis=1)  # local col j = global roll+j
        key_lab = np.roll(sl, -roll)
        ql = sl[qs:qs + Q]                       # own query labels

        wm = np.zeros((128, QT, WIN), dtype=bf16)
        for t in range(QT):
            kl = key_lab[t * 128:t * 128 + WIN]          # [768]
            qlab = ql[t * 128:(t + 1) * 128]             # [128]
            m = (qlab[:, None] == kl[None, :])
            m[np.arange(128), OWN_OFF + np.arange(128)] = False  # self
            wm[:, t, :] = m.astype(bf16)

        pcls = np.zeros((128, QT, C), dtype=bf16)
        qlm = ql.reshape(QT, 128)                # [t, p]
        for t in range(QT):
            pcls[np.arange(128), t, qlm[t]] = bf16(1.0)

        fwinv = np.ascontiguousarray(cfr[qlm].T)  # [128 p, QT t]

        im = {
            "wmask": wm,
            "protosT": protosT,
            "pcls": pcls,
            "cfrT": cfrT,
            "fwinv": fwinv,
            "ones1": ones1,
        }
        for p in range(NPAIR):
            im[f"keys{p}"] = np.ascontiguousarray(
                keysT[:, p * 2048:(p + 1) * 2048])
        in_maps.append(im)
    return in_maps


def run(in_maps, trace=False):
    _install_ntff_hook()
    from concourse import bass_utils

    nc = build_nc()
    res = bass_utils.run_bass_kernel_spmd(
        nc, in_maps, core_ids=list(range(N_CORES)), trace=trace)
    return res


def kernel(protos, proj2, target2, proj3, target3):
    in_maps = make_in_maps(protos, proj2, target2, proj3, target3)
    res = run(in_maps, trace=False)
    parts = [res.results[i]["out"][0, 0] for i in range(N_CORES)]
    total = np.sum(np.asarray(parts, dtype=np.float32))
    return np.asarray(total / np.float32(M), dtype=np.float32)


# revision 9
# speedup vs baseline: 1.7416x; 1.2366x over previous
"""Trainium2 Bass kernel for CropConLoss (supervised-contrastive style loss).

Contract: kernel(**inputs) takes the FULL unsharded inputs
(protos [64,128] f32, proj2/proj3 [4096,128] f32, target2/target3 [4096] i64)
and returns the FULL output (scalar f32 mean loss), running the compute on
8 NeuronCores via bass_utils.run_bass_kernel_spmd.

Strategy (v2.1 — query-partition layout, ACT-roofline design):
  - Host: L2-normalize feats+protos in f32, SORT the 8192 rows by label.
    Core c owns sorted rows [c*1024, (c+1)*1024) as queries. Each core gets
    a cyclically rolled copy of the normalized keys (bf16, [128d x 8192k])
    with its own queries at columns 256..1280, so all same-class keys of
    query (t, p) lie inside the fixed window [t*128, t*128+768) — identical
    control flow on every core (SPMD-safe), per-core data in in_maps.
  - Device main loop (t outer, 4 column groups inner): sim tile
    [128q, 2048k] via 4x 512-col matmuls into ping-pong PSUM; ACT runs
    et = exp(10*sim) back-to-back (the critical path, ~2us per chunk);
    DVE folds the 4 et chunks of a q-tile with 2x-mode adds and one
    1x reduce into the denominator row sums, plus a masked 768-wide
    window reduce for the numerator (mask excludes self).
  - Self-similarity is removed by subtracting exp(10) from the row sum
    (keys are pre-normalized so sim_ii = 1 up to bf16 rounding).
  - Proto terms: 8 packed [128q, 64c] matmuls -> one [128, 512] exp at
    startup; masked flat reduces give numer_proto / denom_proto.
  - Epilogue: denom = (rowsum - e^10)*fwinv + denom_proto + eps;
    loss rows = ln(denom) - ln(numer); free-dim accum + ones-matmul
    partition reduce -> scalar partial per core. Host sums 8 partials /8192.
"""

import sys
import types

sys.path.insert(0, "/opt/trn_rl_repo")

import numpy as np

TAU = 0.1
EPS_FREQ = 1e-06
EPS_DENOM = 1e-12

N_CORES = 8
M = 8192           # total rows (2*4096)
D = 128            # feature dim
C = 64             # num classes
Q = M // N_CORES   # 1024 query rows per core
QT = Q // 128      # 8 query tiles per core
NPAIR = 4          # 4 column groups of 2048 keys
WIN = 768          # numer window width (covers class runs up to 257)
OWN_OFF = 256      # own queries start at this column of the rolled buffer
E10 = float(np.exp(np.float64(1.0 / TAU)))  # 22026.4657948...


def _install_ntff_hook():
    """Shim antenv.axon_hooks (absent in this image) so trace=True works."""
    if "antenv.axon_hooks" in sys.modules:
        return
    try:
        if "/root/.axon_site" not in sys.path:
            sys.path.insert(0, "/root/.axon_site")
        import trn_agent_boot.trn_boot as tb

        hook = tb._ntff_profile_via_ctypes("/opt/axon/libaxon_pjrt.so")
        mod = types.ModuleType("antenv.axon_hooks")
        mod._hook = hook
        mod.get_axon_ntff_profile_hook = lambda: mod._hook
        mod.set_axon_ntff_profile_hook = lambda h: setattr(mod, "_hook", h)
        sys.modules["antenv.axon_hooks"] = mod
        import antenv

        antenv.axon_hooks = mod
    except Exception:
        pass


def build_nc():
    """Build and compile the single-core Bass program (same NEFF on all 8)."""
    import concourse.bass as bass  # noqa: F401
    import concourse.mybir as mybir
    import concourse.bacc as bacc
    from concourse import tile

    f32 = mybir.dt.float32
    bf16 = mybir.dt.bfloat16
    mult = mybir.AluOpType.mult
    add = mybir.AluOpType.add
    sub = mybir.AluOpType.subtract
    Act = mybir.ActivationFunctionType

    nc = bacc.Bacc("TRN2", target_bir_lowering=False, debug=False,
                   num_devices=N_CORES)

    # DRAM I/O (per-core data via in_maps). keysT split in 4 column groups
    # so the first matmuls depend only on the first 512KB DMA.
    d_keys = [nc.dram_tensor(f"keys{p}", [128, 2048], bf16,
                             kind="ExternalInput") for p in range(NPAIR)]
    d_wmask = nc.dram_tensor("wmask", [128, QT, WIN], bf16,
                             kind="ExternalInput")
    d_protosT = nc.dram_tensor("protosT", [128, C], bf16,
                               kind="ExternalInput")
    d_pclsf = nc.dram_tensor("pclsf", [128, QT * C], f32,
                             kind="ExternalInput")
    d_cfrf = nc.dram_tensor("cfrf", [128, QT * C], f32, kind="ExternalInput")
    d_fwinv = nc.dram_tensor("fwinv", [128, QT], f32, kind="ExternalInput")
    d_ones = nc.dram_tensor("ones1", [128, 1], f32, kind="ExternalInput")
    d_out = nc.dram_tensor("out", [1, 1], f32, kind="ExternalOutput")

    with tile.TileContext(nc) as tc:
        with (
            tc.tile_pool(name="const", bufs=1) as cst,
            tc.tile_pool(name="work", bufs=3) as work,
            tc.tile_pool(name="etring", bufs=6) as etring,
            tc.tile_pool(name="accring", bufs=2) as accring,
            tc.tile_pool(name="dscr", bufs=2) as dscr,
            tc.tile_pool(name="wscr", bufs=2) as wscr,
        ):
            # ---- resident SBUF tensors ----
            keys = [cst.tile([128, 2048], bf16, name=f"keys_sb{p}",
                             tag=f"keys{p}") for p in range(NPAIR)]
            wmask = cst.tile([128, QT, WIN], bf16, tag="wmask")
            protosT = cst.tile([128, C], bf16, tag="protosT")
            pclsf = cst.tile([128, QT * C], f32, tag="pclsf")
            cfrf = cst.tile([128, QT * C], f32, tag="cfrf")
            fwinv = cst.tile([128, QT], f32, tag="fwinv")
            ones1 = cst.tile([128, 1], f32, tag="ones1")

            npro = cst.tile([128, QT], f32, tag="npro")
            dpro = cst.tile([128, QT], f32, tag="dpro")
            nmr = cst.tile([128, QT], f32, tag="nmr")
            dend = cst.tile([128, QT], f32, tag="dend")

            # ---- DMAs in priority order ----
            nc.sync.dma_start(protosT[:], d_protosT[:])
            nc.sync.dma_start(pclsf[:], d_pclsf[:])
            nc.sync.dma_start(cfrf[:], d_cfrf[:])
            nc.sync.dma_start(fwinv[:], d_fwinv[:])
            nc.sync.dma_start(ones1[:], d_ones[:])
            for p in range(NPAIR):
                nc.sync.dma_start(keys[p][:], d_keys[p][:])
            nc.sync.dma_start(wmask[:], d_wmask[:])

            def qstat(t):
                # stationary for q-tile t: own queries at cols 256..1280
                lo = OWN_OFF + t * 128
                return keys[0][:, lo:lo + 128]

            # ---- proto phase: 8 packed [128q, 64c] -> one exp ----
            with tc.tile_pool(name="pp", bufs=1, space="PSUM") as ppp:
                ps0 = ppp.tile([128, QT * C], f32, tag="ps0")
                for t in range(QT):
                    nc.tensor.matmul(ps0[:, t * C:(t + 1) * C], qstat(t),
                                     protosT[:], start=True, stop=True)
                pe = cst.tile([128, QT * C], f32, tag="pe")
                nc.scalar.activation(pe[:], ps0[:], Act.Exp,
                                     scale=1.0 / TAU)
            pn_ = work.tile([128, QT * C], f32, tag="pn_")
            nc.vector.tensor_tensor(pn_[:], pe[:], pclsf[:], op=mult)
            pd_ = work.tile([128, QT * C], f32, tag="pd_")
            nc.vector.tensor_tensor(pd_[:], pe[:], cfrf[:], op=mult)
            for t in range(QT):
                s1 = wscr.tile([128, C], f32, tag="s1")
                nc.vector.tensor_scalar(
                    s1[:], pn_[:, t * C:(t + 1) * C], 1.0, None, op0=mult,
                    op1=add, accum_out=npro[:, t:t + 1])
                s2 = wscr.tile([128, C], f32, tag="s2")
                nc.vector.tensor_scalar(
                    s2[:], pd_[:, t * C:(t + 1) * C], 1.0, None, op0=mult,
                    op1=add, accum_out=dpro[:, t:t + 1])

            with tc.tile_pool(name="ps", bufs=2, space="PSUM") as psp:
                # ---- main loop: 8 q-tiles x 4 column groups ----
                for t in range(QT):
                    ets = []
                    acc = None
                    for p in range(NPAIR):
                        ps = psp.tile([128, 2048], f32, tag="ps")
                        for j in range(4):
                            nc.tensor.matmul(
                                ps[:, j * 512:(j + 1) * 512], qstat(t),
                                keys[p][:, j * 512:(j + 1) * 512],
                                start=True, stop=True)
                        et = etring.tile([128, 2048], bf16, tag="et")
                        nc.scalar.activation(et[:], ps[:], Act.Exp,
                                             scale=1.0 / TAU)
                        ets.append(et)
                        if p == 1:
                            acc = accring.tile([128, 2048], bf16, tag="acc")
                            nc.vector.tensor_tensor(acc[:], ets[0][:],
                                                    ets[1][:], op=add)
                        elif p > 1:
                            nc.vector.tensor_tensor(acc[:], acc[:], et[:],
                                                    op=add)
                    # denominator row sum for this q-tile
                    dsc = dscr.tile([128, 2048], bf16, tag="dsc")
                    nc.vector.tensor_scalar(
                        dsc[:], acc[:], 1.0, None, op0=mult, op1=add,
                        accum_out=dend[:, t:t + 1])
                    # numerator: masked window reduce on the p=0 chunk
                    wsc = wscr.tile([128, WIN], bf16, tag="wsc")
                    nc.vector.tensor_tensor(
                        wsc[:], ets[0][:, t * 128:t * 128 + WIN],
                        wmask[:, t], op=mult)
                    wsc2 = wscr.tile([128, WIN], bf16, tag="wsc2")
                    nc.vector.tensor_scalar(
                        wsc2[:], wsc[:], 1.0, None, op0=mult, op1=add,
                        accum_out=nmr[:, t:t + 1])

            # ---- epilogue ----
            den = cst.tile([128, QT], f32, tag="den")
            nc.vector.tensor_scalar_add(den[:], dend[:], -E10)
            nc.vector.tensor_tensor(den[:], den[:], fwinv[:], op=mult)
            nc.vector.tensor_tensor(den[:], den[:], dpro[:], op=add)
            nc.vector.tensor_scalar_add(den[:], den[:], EPS_DENOM)
            nc.vector.tensor_tensor(nmr[:], nmr[:], npro[:], op=add)

            lden = work.tile([128, QT], f32, tag="lden")
            ld = cst.tile([128, 1], f32, tag="ld")
            nc.scalar.activation(lden[:], den[:], Act.Ln, accum_out=ld[:])
            lnum = work.tile([128, QT], f32, tag="lnum")
            ln_ = cst.tile([128, 1], f32, tag="ln_")
            nc.scalar.activation(lnum[:], nmr[:], Act.Ln, accum_out=ln_[:])
            diff = cst.tile([128, 1], f32, tag="diff")
            nc.vector.tensor_tensor(diff[:], ld[:], ln_[:], op=sub)

            with tc.tile_pool(name="rp", bufs=1, space="PSUM") as rp:
                res_ps = rp.tile([1, 1], f32, tag="res_ps")
                nc.tensor.matmul(res_ps[:], ones1[:], diff[:],
                                 start=True, stop=True)
                res = cst.tile([1, 1], f32, tag="res")
                nc.vector.tensor_copy(res[:], res_ps[:])
                nc.sync.dma_start(d_out[:], res[:])

    nc.compile()
    return nc


def make_in_maps(protos, proj2, target2, proj3, target3):
    import ml_dtypes

    bf16 = ml_dtypes.bfloat16
    f32 = np.float32

    feats = np.concatenate([np.asarray(proj2, dtype=f32),
                            np.asarray(proj3, dtype=f32)], axis=0)
    labels = np.concatenate([np.asarray(target2), np.asarray(target3)],
                            axis=0).astype(np.int64)

    # f32 normalization (matches reference F.normalize)
    nrm = np.sqrt(np.sum(feats.astype(f32) ** 2, axis=1, keepdims=True,
                         dtype=f32))
    fn = feats / np.maximum(nrm, f32(1e-12))
    pr = np.asarray(protos, dtype=f32)
    pnrm = np.sqrt(np.sum(pr ** 2, axis=1, keepdims=True, dtype=f32))
    pn = pr / np.maximum(pnrm, f32(1e-12))

    counts = np.bincount(labels, minlength=C).astype(f32)
    cls_freq = (counts + f32(1.0)) + f32(EPS_FREQ)
    cfr = (f32(1.0) / cls_freq).astype(f32)

    perm = np.argsort(labels, kind="stable")
    sf = np.ascontiguousarray(fn[perm])          # [8192, 128] sorted by label
    sl = labels[perm]                            # [8192]
    assert counts.max() <= 257, f"class run too long: {counts.max()}"

    keysT_g = np.ascontiguousarray(sf.T).astype(bf16)   # [128, 8192]
    protosT = np.ascontiguousarray(pn.T).astype(bf16)   # [128, 64]
    ones1 = np.ones((128, 1), dtype=f32)
    # cfr flat: [128, QT*C], same for every (p, t)
    cfrf = np.broadcast_to(np.tile(cfr, QT)[None, :],
                           (128, QT * C)).astype(f32).copy()

    in_maps = []
    for c in range(N_CORES):
        qs = c * Q
        roll = (qs - OWN_OFF) % M
        keysT = np.roll(keysT_g, -roll, axis=1)  # local col j = global roll+j
        key_lab = np.roll(sl, -roll)
        ql = sl[qs:qs + Q]                       # own query labels

        wm = np.zeros((128, QT, WIN), dtype=bf16)
        for t in range(QT):
            kl = key_lab[t * 128:t * 128 + WIN]          # [768]
            qlab = ql[t * 128:(t + 1) * 128]             # [128]
            m = (qlab[:, None] == kl[None, :])
            m[np.arange(128), OWN_OFF + np.arange(128)] = False  # self
            wm[:, t, :] = m.astype(bf16)

        qlm = ql.reshape(QT, 128)                # [t, p]
        pclsf = np.zeros((128, QT * C), dtype=f32)
        for t in range(QT):
            pclsf[np.arange(128), t * C + qlm[t]] = f32(1.0)
        fwinv = np.ascontiguousarray(cfr[qlm].T)  # [128 p, QT t]

        im = {
            "wmask": wm,
            "protosT": protosT,
            "pclsf": pclsf,
            "cfrf": cfrf,
            "fwinv": fwinv,
            "ones1": ones1,
        }
        for p in range(NPAIR):
            im[f"keys{p}"] = np.ascontiguousarray(
                keysT[:, p * 2048:(p + 1) * 2048])
        in_maps.append(im)
    return in_maps


def run(in_maps, trace=False):
    _install_ntff_hook()
    from concourse import bass_utils

    nc = build_nc()
    res = bass_utils.run_bass_kernel_spmd(
        nc, in_maps, core_ids=list(range(N_CORES)), trace=trace)
    return res


def kernel(protos, proj2, target2, proj3, target3):
    in_maps = make_in_maps(protos, proj2, target2, proj3, target3)
    res = run(in_maps, trace=False)
    parts = [res.results[i]["out"][0, 0] for i in range(N_CORES)]
    total = np.sum(np.asarray(parts, dtype=np.float32))
    return np.asarray(total / np.float32(M), dtype=np.float32)


# revision 11
# speedup vs baseline: 1.8466x; 1.0603x over previous
"""Trainium2 Bass kernel for CropConLoss (supervised-contrastive style loss).

Contract: kernel(**inputs) takes the FULL unsharded inputs
(protos [64,128] f32, proj2/proj3 [4096,128] f32, target2/target3 [4096] i64)
and returns the FULL output (scalar f32 mean loss), running the compute on
8 NeuronCores via bass_utils.run_bass_kernel_spmd.

Strategy (v2.1 — query-partition layout, ACT-roofline design):
  - Host: L2-normalize feats+protos in f32, SORT the 8192 rows by label.
    Core c owns sorted rows [c*1024, (c+1)*1024) as queries. Each core gets
    a cyclically rolled copy of the normalized keys (bf16, [128d x 8192k])
    with its own queries at columns 256..1280, so all same-class keys of
    query (t, p) lie inside the fixed window [t*128, t*128+768) — identical
    control flow on every core (SPMD-safe), per-core data in in_maps.
  - Device main loop (t outer, 4 column groups inner): sim tile
    [128q, 2048k] via 4x 512-col matmuls into ping-pong PSUM; ACT runs
    et = exp(10*sim) back-to-back (the critical path, ~2us per chunk);
    DVE folds the 4 et chunks of a q-tile with 2x-mode adds and one
    1x reduce into the denominator row sums, plus a masked 768-wide
    window reduce for the numerator (mask excludes self).
  - Self-similarity is removed by subtracting exp(10) from the row sum
    (keys are pre-normalized so sim_ii = 1 up to bf16 rounding).
  - Proto terms: 8 packed [128q, 64c] matmuls -> one [128, 512] exp at
    startup; masked flat reduces give numer_proto / denom_proto.
  - Epilogue: denom = (rowsum - e^10)*fwinv + denom_proto + eps;
    loss rows = ln(denom) - ln(numer); free-dim accum + ones-matmul
    partition reduce -> scalar partial per core. Host sums 8 partials /8192.
"""

import sys
import types

sys.path.insert(0, "/opt/trn_rl_repo")

import numpy as np

TAU = 0.1
EPS_FREQ = 1e-06
EPS_DENOM = 1e-12

N_CORES = 8
M = 8192           # total rows (2*4096)
D = 128            # feature dim
C = 64             # num classes
Q = M // N_CORES   # 1024 query rows per core
QT = Q // 128      # 8 query tiles per core
NPAIR = 4          # 4 column groups of 2048 keys
WIN = 768          # numer window width (covers class runs up to 257)
OWN_OFF = 256      # own queries start at this column of the rolled buffer
E10 = float(np.exp(np.float64(1.0 / TAU)))  # 22026.4657948...


def _install_ntff_hook():
    """Shim antenv.axon_hooks (absent in this image) so trace=True works."""
    if "antenv.axon_hooks" in sys.modules:
        return
    try:
        if "/root/.axon_site" not in sys.path:
            sys.path.insert(0, "/root/.axon_site")
        import trn_agent_boot.trn_boot as tb

        hook = tb._ntff_profile_via_ctypes("/opt/axon/libaxon_pjrt.so")
        mod = types.ModuleType("antenv.axon_hooks")
        mod._hook = hook
        mod.get_axon_ntff_profile_hook = lambda: mod._hook
        mod.set_axon_ntff_profile_hook = lambda h: setattr(mod, "_hook", h)
        sys.modules["antenv.axon_hooks"] = mod
        import antenv

        antenv.axon_hooks = mod
    except Exception:
        pass


def build_nc():
    """Build and compile the single-core Bass program (same NEFF on all 8)."""
    import concourse.bass as bass  # noqa: F401
    import concourse.mybir as mybir
    import concourse.bacc as bacc
    from concourse import tile

    f32 = mybir.dt.float32
    bf16 = mybir.dt.bfloat16
    mult = mybir.AluOpType.mult
    add = mybir.AluOpType.add
    sub = mybir.AluOpType.subtract
    Act = mybir.ActivationFunctionType

    nc = bacc.Bacc("TRN2", target_bir_lowering=False, debug=False,
                   num_devices=N_CORES)

    # DRAM I/O (per-core data via in_maps). keysT split in 4 column groups
    # so the first matmuls depend only on the first 512KB DMA.
    d_keys = [nc.dram_tensor(f"keys{p}", [128, 2048], bf16,
                             kind="ExternalInput") for p in range(NPAIR)]
    d_wmask = nc.dram_tensor("wmask", [128, QT, WIN], bf16,
                             kind="ExternalInput")
    d_protosT = nc.dram_tensor("protosT", [128, C], bf16,
                               kind="ExternalInput")
    d_pclsf = nc.dram_tensor("pclsf", [128, QT * C], bf16,
                             kind="ExternalInput")
    d_cfrf = nc.dram_tensor("cfrf", [128, QT * C], bf16, kind="ExternalInput")
    d_fwinv = nc.dram_tensor("fwinv", [128, QT], f32, kind="ExternalInput")
    d_ones = nc.dram_tensor("ones1", [128, 1], f32, kind="ExternalInput")
    d_out = nc.dram_tensor("out", [1, 1], f32, kind="ExternalOutput")

    with tile.TileContext(nc) as tc:
        with (
            tc.tile_pool(name="const", bufs=1) as cst,
            tc.tile_pool(name="work", bufs=3) as work,
            tc.tile_pool(name="etring", bufs=6) as etring,
            tc.tile_pool(name="accring", bufs=2) as accring,
            tc.tile_pool(name="dscr", bufs=2) as dscr,
            tc.tile_pool(name="wscr", bufs=2) as wscr,
        ):
            # ---- resident SBUF tensors ----
            keys = [cst.tile([128, 2048], bf16, name=f"keys_sb{p}",
                             tag=f"keys{p}") for p in range(NPAIR)]
            wmask = cst.tile([128, QT, WIN], bf16, tag="wmask")
            protosT = cst.tile([128, C], bf16, tag="protosT")
            pclsf = cst.tile([128, QT * C], bf16, tag="pclsf")
            cfrf = cst.tile([128, QT * C], bf16, tag="cfrf")
            fwinv = cst.tile([128, QT], f32, tag="fwinv")
            ones1 = cst.tile([128, 1], f32, tag="ones1")

            npro = cst.tile([128, QT], f32, tag="npro")
            dpro = cst.tile([128, QT], f32, tag="dpro")
            nmr = cst.tile([128, QT], f32, tag="nmr")
            dend = cst.tile([128, QT], f32, tag="dend")
            acc3 = cst.tile([128, QT], f32, tag="acc3")
            e7 = cst.tile([128, NPAIR], f32, tag="e7")

            # ---- DMAs in priority order ----
            nc.sync.dma_start(protosT[:], d_protosT[:])
            for p in range(NPAIR):
                nc.sync.dma_start(keys[p][:], d_keys[p][:])
            nc.sync.dma_start(pclsf[:], d_pclsf[:])
            nc.sync.dma_start(cfrf[:], d_cfrf[:])
            nc.sync.dma_start(fwinv[:], d_fwinv[:])
            nc.sync.dma_start(ones1[:], d_ones[:])
            nc.sync.dma_start(wmask[:], d_wmask[:])

            def qstat(t):
                # stationary for q-tile t: own queries at cols 256..1280
                lo = OWN_OFF + t * 128
                return keys[0][:, lo:lo + 128]

            with tc.tile_pool(name="ps", bufs=2, space="PSUM") as psp:
                # ---- proto phase: 8 packed [128q, 64c] -> one exp ----
                ps0 = psp.tile([128, 2048], f32, tag="ps")
                for t in range(QT):
                    nc.tensor.matmul(ps0[:, t * C:(t + 1) * C], qstat(t),
                                     protosT[:], start=True, stop=True)
                pe = cst.tile([128, QT * C], bf16, tag="pe")
                nc.scalar.activation(pe[:], ps0[:, 0:QT * C], Act.Exp,
                                     scale=1.0 / TAU)
                nc.vector.memset(acc3[:], 0.0)
                pn_ = work.tile([128, QT * C], bf16, tag="pn_")
                nc.vector.tensor_tensor(pn_[:], pe[:], pclsf[:], op=mult)
                pd_ = work.tile([128, QT * C], bf16, tag="pd_")
                nc.vector.tensor_tensor(pd_[:], pe[:], cfrf[:], op=mult)
                for t in range(QT):
                    s1 = wscr.tile([128, C], bf16, tag="s1")
                    nc.vector.tensor_scalar(
                        s1[:], pn_[:, t * C:(t + 1) * C], 1.0, None, op0=mult,
                        op1=add, accum_out=npro[:, t:t + 1])
                    s2 = wscr.tile([128, C], bf16, tag="s2")
                    nc.vector.tensor_scalar(
                        s2[:], pd_[:, t * C:(t + 1) * C], 1.0, None, op0=mult,
                        op1=add, accum_out=dpro[:, t:t + 1])

                # ---- main loop: 8 q-tiles x 4 column groups ----
                for t in range(QT):
                    last = t == QT - 1
                    ets = []
                    acc = None
                    for p in range(NPAIR):
                        ps = psp.tile([128, 2048], f32, tag="ps")
                        for j in range(4):
                            nc.tensor.matmul(
                                ps[:, j * 512:(j + 1) * 512], qstat(t),
                                keys[p][:, j * 512:(j + 1) * 512],
                                start=True, stop=True)
                        et = etring.tile([128, 2048], bf16, tag="et")
                        if last:
                            # last q-tile: ACT accumulates every chunk, so
                            # no big DVE reduce sits on the epilogue path
                            nc.scalar.activation(et[:], ps[:], Act.Exp,
                                                 scale=1.0 / TAU,
                                                 accum_out=e7[:, p:p + 1])
                        elif p == NPAIR - 1:
                            nc.scalar.activation(et[:], ps[:], Act.Exp,
                                                 scale=1.0 / TAU,
                                                 accum_out=acc3[:, t:t + 1])
                        else:
                            nc.scalar.activation(et[:], ps[:], Act.Exp,
                                                 scale=1.0 / TAU)
                        ets.append(et)
                        if not last:
                            if p == 1:
                                acc = accring.tile([128, 2048], bf16,
                                                   tag="acc")
                                nc.vector.tensor_tensor(acc[:], ets[0][:],
                                                        ets[1][:], op=add)
                            elif p == 2:
                                nc.vector.tensor_tensor(acc[:], acc[:],
                                                        et[:], op=add)
                    if last:
                        es = wscr.tile([128, NPAIR], f32, tag="es")
                        nc.vector.tensor_scalar(
                            es[:], e7[:], 1.0, None, op0=mult, op1=add,
                            accum_out=dend[:, t:t + 1])
                    else:
                        # denominator row sum over chunks 0..2 (chunk 3 was
                        # accumulated by ACT into acc3)
                        dsc = dscr.tile([128, 2048], bf16, tag="dsc")
                        nc.vector.tensor_scalar(
                            dsc[:], acc[:], 1.0, None, op0=mult, op1=add,
                            accum_out=dend[:, t:t + 1])
                    # numerator: masked window reduce on the p=0 chunk
                    wsc = wscr.tile([128, WIN], bf16, tag="wsc")
                    nc.vector.tensor_tensor(
                        wsc[:], ets[0][:, t * 128:t * 128 + WIN],
                        wmask[:, t], op=mult)
                    wsc2 = wscr.tile([128, WIN], bf16, tag="wsc2")
                    nc.vector.tensor_scalar(
                        wsc2[:], wsc[:], 1.0, None, op0=mult, op1=add,
                        accum_out=nmr[:, t:t + 1])

            # ---- epilogue ----
            den = cst.tile([128, QT], f32, tag="den")
            nc.vector.tensor_tensor(den[:], dend[:], acc3[:], op=add)
            nc.vector.tensor_scalar_add(den[:], den[:], -E10)
            nc.vector.tensor_tensor(den[:], den[:], fwinv[:], op=mult)
            nc.vector.tensor_tensor(den[:], den[:], dpro[:], op=add)
            nc.vector.tensor_scalar_add(den[:], den[:], EPS_DENOM)
            nc.vector.tensor_tensor(nmr[:], nmr[:], npro[:], op=add)

            lden = work.tile([128, QT], f32, tag="lden")
            ld = cst.tile([128, 1], f32, tag="ld")
            nc.scalar.activation(lden[:], den[:], Act.Ln, accum_out=ld[:])
            lnum = work.tile([128, QT], f32, tag="lnum")
            ln_ = cst.tile([128, 1], f32, tag="ln_")
            nc.scalar.activation(lnum[:], nmr[:], Act.Ln, accum_out=ln_[:])
            diff = cst.tile([128, 1], f32, tag="diff")
            nc.vector.tensor_tensor(diff[:], ld[:], ln_[:], op=sub)

            with tc.tile_pool(name="rp", bufs=1, space="PSUM") as rp:
                res_ps = rp.tile([1, 1], f32, tag="res_ps")
                nc.tensor.matmul(res_ps[:], ones1[:], diff[:],
                                 start=True, stop=True)
                res = cst.tile([1, 1], f32, tag="res")
                nc.vector.tensor_copy(res[:], res_ps[:])
                nc.sync.dma_start(d_out[:], res[:])

    nc.compile()
    return nc


def make_in_maps(protos, proj2, target2, proj3, target3):
    import ml_dtypes

    bf16 = ml_dtypes.bfloat16
    f32 = np.float32

    feats = np.concatenate([np.asarray(proj2, dtype=f32),
                            np.asarray(proj3, dtype=f32)], axis=0)
    labels = np.concatenate([np.asarray(target2), np.asarray(target3)],
                            axis=0).astype(np.int64)

    # f32 normalization (matches reference F.normalize)
    nrm = np.sqrt(np.sum(feats.astype(f32) ** 2, axis=1, keepdims=True,
                         dtype=f32))
    fn = feats / np.maximum(nrm, f32(1e-12))
    pr = np.asarray(protos, dtype=f32)
    pnrm = np.sqrt(np.sum(pr ** 2, axis=1, keepdims=True, dtype=f32))
    pn = pr / np.maximum(pnrm, f32(1e-12))

    counts = np.bincount(labels, minlength=C).astype(f32)
    cls_freq = (counts + f32(1.0)) + f32(EPS_FREQ)
    cfr = (f32(1.0) / cls_freq).astype(f32)

    perm = np.argsort(labels, kind="stable")
    sf = np.ascontiguousarray(fn[perm])          # [8192, 128] sorted by label
    sl = labels[perm]                            # [8192]
    assert counts.max() <= 257, f"class run too long: {counts.max()}"

    keysT_g = np.ascontiguousarray(sf.T).astype(bf16)   # [128, 8192]
    protosT = np.ascontiguousarray(pn.T).astype(bf16)   # [128, 64]
    ones1 = np.ones((128, 1), dtype=f32)
    # cfr flat: [128, QT*C], same for every (p, t)
    cfrf = np.broadcast_to(np.tile(cfr, QT)[None, :],
                           (128, QT * C)).astype(bf16).copy()

    in_maps = []
    for c in range(N_CORES):
        qs = c * Q
        roll = (qs - OWN_OFF) % M
        keysT = np.roll(keysT_g, -roll, axis=1)  # local col j = global roll+j
        key_lab = np.roll(sl, -roll)
        ql = sl[qs:qs + Q]                       # own query labels

        wm = np.zeros((128, QT, WIN), dtype=bf16)
        for t in range(QT):
            kl = key_lab[t * 128:t * 128 + WIN]          # [768]
            qlab = ql[t * 128:(t + 1) * 128]             # [128]
            m = (qlab[:, None] == kl[None, :])
            m[np.arange(128), OWN_OFF + np.arange(128)] = False  # self
            wm[:, t, :] = m.astype(bf16)

        qlm = ql.reshape(QT, 128)                # [t, p]
        pclsf = np.zeros((128, QT * C), dtype=bf16)
        for t in range(QT):
            pclsf[np.arange(128), t * C + qlm[t]] = bf16(1.0)
        fwinv = np.ascontiguousarray(cfr[qlm].T)  # [128 p, QT t]

        im = {
            "wmask": wm,
            "protosT": protosT,
            "pclsf": pclsf,
            "cfrf": cfrf,
            "fwinv": fwinv,
            "ones1": ones1,
        }
        for p in range(NPAIR):
            im[f"keys{p}"] = np.ascontiguousarray(
                keysT[:, p * 2048:(p + 1) * 2048])
        in_maps.append(im)
    return in_maps


def run(in_maps, trace=False):
    _install_ntff_hook()
    from concourse import bass_utils

    nc = build_nc()
    res = bass_utils.run_bass_kernel_spmd(
        nc, in_maps, core_ids=list(range(N_CORES)), trace=trace)
    return res


def kernel(protos, proj2, target2, proj3, target3):
    in_maps = make_in_maps(protos, proj2, target2, proj3, target3)
    res = run(in_maps, trace=False)
    parts = [res.results[i]["out"][0, 0] for i in range(N_CORES)]
    total = np.sum(np.asarray(parts, dtype=np.float32))
    return np.asarray(total / np.float32(M), dtype=np.float32)
